# revision 5
# baseline (speedup 1.0000x reference)
"""Multi-head attention (B=2, S=2048, D=1024, H=16, DH=64) on 8 TRN2 cores.

Sharding: core c handles batch b = c//4 and head group g = c%4 (4 heads).
Each core computes, for its (b, g):
    QhT/KhT/VhT = per-head projections (transposed layout [e, s]),
    S^T = Kh @ Qh^T per head (scores transposed, j on partitions),
    P^T = exp(S^T / sqrt(dk))  (no max subtraction; fp32 range is ample),
    outT_ext = Vh_ext^T @ P^T  (row 64 = softmax denominators via ones col),
    outT = outT_unnorm * (1/l) broadcast,
    PT_partial = Wf_slice^T-contract -> partial final projection [D, S].
Host: out[b] = (sum_g PT_partial).T + bf.

All matmuls run in float32r (fp32 with low 12 mantissa bits truncated) which
streams at full PE rate (1 cycle/row) instead of fp32's 4 cycles/row.
Inputs consumed by matmuls are pre-truncated on the host and DMA'd into
float32r tiles; on-device f32r producers are DVE/ACT ops with f32r outputs.
"""

import sys

sys.path.insert(0, "/opt/trn_rl_repo")

from contextlib import ExitStack

import numpy as np

import concourse.mybir as mybir
import concourse.tile as tile
from concourse import bacc
from concourse.bass_utils import run_bass_kernel_spmd
from concourse.masks import make_identity

B, S, D, H, DH = 2, 2048, 1024, 16, 64
NCORES = 8
GPB = 4  # head-group cores per batch
HPG = H // GPB  # heads per group (4)
CW = HPG * DH  # concat width per core (256)
NPAIR = HPG // 2  # head pairs per group (2)
DCH = D // 128  # d chunks (8)
JCH = S // 128  # key chunks (16)
IB = 1024  # i-block width for attention
NIB = S // IB  # 2
F32 = mybir.dt.float32
F32R = mybir.dt.float32r
AF = mybir.ActivationFunctionType
INV_SQRT_DK = 1.0 / np.sqrt(DH)

_CACHE = {}


def _chop(x):
    """Truncate fp32 to float32r bit format (low 12 mantissa bits zeroed)."""
    a = np.ascontiguousarray(x, dtype=np.float32)
    return (a.view(np.uint32) & ~np.uint32(0xFFF)).view(np.float32)


def _build():
    nc = bacc.Bacc("TRN2", target_bir_lowering=False, debug=False, num_devices=NCORES)

    qt_d = nc.dram_tensor("qt", [D, S], F32R, kind="ExternalInput").ap()
    kt_d = nc.dram_tensor("kt", [D, S], F32R, kind="ExternalInput").ap()
    vt_d = nc.dram_tensor("vt", [D, S], F32R, kind="ExternalInput").ap()
    wq_d = nc.dram_tensor("wq", [D, CW], F32R, kind="ExternalInput").ap()
    wk_d = nc.dram_tensor("wk", [D, CW], F32R, kind="ExternalInput").ap()
    wv_d = nc.dram_tensor("wv", [D, CW], F32R, kind="ExternalInput").ap()
    wf_d = nc.dram_tensor("wf", [CW, D], F32R, kind="ExternalInput").ap()
    bq_d = nc.dram_tensor("bq", [CW], F32, kind="ExternalInput").ap()
    bk_d = nc.dram_tensor("bk", [CW], F32, kind="ExternalInput").ap()
    bv_d = nc.dram_tensor("bv", [CW], F32, kind="ExternalInput").ap()
    ones_col_d = nc.dram_tensor("ones_col", [128, JCH, 1], F32R, kind="ExternalInput").ap()
    ones_row_d = nc.dram_tensor("ones_row", [1, 64], F32R, kind="ExternalInput").ap()
    pt_d = nc.dram_tensor("pt", [D, S], F32, kind="ExternalOutput").ap()

    with (
        tile.TileContext(nc) as tc,
        nc.allow_low_precision(reason="float32r matmul inputs are intentional"),
        ExitStack() as ctx,
    ):
        const = ctx.enter_context(tc.tile_pool(name="const", bufs=1))
        persist = ctx.enter_context(tc.tile_pool(name="persist", bufs=1))

        # ---- constants ----
        wq_sb = const.tile([128, DCH * CW], F32R, tag="wq")
        wk_sb = const.tile([128, DCH * CW], F32R, tag="wk")
        wv_sb = const.tile([128, DCH * CW], F32R, tag="wv")
        wf_sb = const.tile([128, 2 * D], F32R, tag="wf")
        bq_sb = const.tile([128, NPAIR], F32, tag="bq")
        bk_sb = const.tile([128, NPAIR], F32, tag="bk")
        bv_sb = const.tile([128, NPAIR], F32, tag="bv")
        ident = const.tile([128, 128], F32, tag="ident")
        ones64 = const.tile([1, 64], F32R, tag="ones")

        for w_sb, w_dram in ((wq_sb, wq_d), (wk_sb, wk_d), (wv_sb, wv_d)):
            nc.sync.dma_start(
                out=w_sb[:].rearrange("p (c e) -> p c e", c=DCH),
                in_=w_dram.rearrange("(c p) e -> p c e", p=128),
            )
        nc.sync.dma_start(
            out=wf_sb[:].rearrange("p (c f) -> p c f", c=2),
            in_=wf_d.rearrange("(c p) f -> p c f", p=128),
        )
        for b_sb, b_dram in ((bq_sb, bq_d), (bk_sb, bk_d), (bv_sb, bv_d)):
            nc.sync.dma_start(
                out=b_sb[:], in_=b_dram.rearrange("(r p) -> p r", p=128)
            )
        make_identity(nc, ident[:])
        nc.sync.dma_start(out=ones64[:], in_=ones_row_d)

        # ---- persistent activations ----
        qhT = [persist.tile([128, S], F32R, tag=f"qhT{r}", name=f"qhT{r}") for r in range(NPAIR)]
        khT = [persist.tile([128, S], F32R, tag=f"khT{r}", name=f"khT{r}") for r in range(NPAIR)]
        vhe = [persist.tile([128, JCH * 65], F32R, tag=f"vhe{h}", name=f"vhe{h}") for h in range(HPG)]
        outT = [persist.tile([128, S], F32R, tag=f"outT{r}", name=f"outT{r}") for r in range(NPAIR)]
        for h in range(HPG):
            nc.sync.dma_start(
                out=vhe[h][:].rearrange("p (c w) -> p c w", w=65)[:, :, 64:65],
                in_=ones_col_d,
            )

        # ---- phase 1: projections ----
        with (
            tc.tile_pool(name="xt", bufs=3) as xt_pool,
            tc.tile_pool(name="vtmp", bufs=1) as vtmp_pool,
        ):
            vhT_tmp = [None, None]
            with tc.tile_pool(name="ps_proj", bufs=2, space="PSUM") as ps_proj:
                for x_dram, w_sb, b_sb, kind in (
                    (qt_d, wq_sb, bq_sb, "Q"),
                    (kt_d, wk_sb, bk_sb, "K"),
                    (vt_d, wv_sb, bv_sb, "V"),
                ):
                    ps = [ps_proj.tile([128, S], F32, tag="proj", name="ps_proj_t") for _ in range(NPAIR)]
                    for d in range(DCH):
                        xt = xt_pool.tile([128, S], F32R, tag="xt")
                        nc.sync.dma_start(out=xt[:], in_=x_dram[128 * d : 128 * (d + 1), :])
                        for r in range(NPAIR):
                            w_st = w_sb[:, CW * d + 128 * r : CW * d + 128 * (r + 1)]
                            for s4 in range(S // 512):
                                nc.tensor.matmul(
                                    ps[r][:, 512 * s4 : 512 * (s4 + 1)],
                                    w_st,
                                    xt[:, 512 * s4 : 512 * (s4 + 1)],
                                    start=(d == 0),
                                    stop=(d == DCH - 1),
                                )
                    for r in range(NPAIR):
                        if kind == "Q":
                            nc.vector.tensor_scalar_add(qhT[r][:], ps[r][:], b_sb[:, r : r + 1])
                        elif kind == "K":
                            nc.vector.tensor_scalar_add(khT[r][:], ps[r][:], b_sb[:, r : r + 1])
                        else:
                            vhT_tmp[r] = vtmp_pool.tile([128, S], F32, tag=f"vtmp{r}", name=f"vtmp{r}")
                            nc.vector.tensor_scalar_add(
                                vhT_tmp[r][:], ps[r][:], b_sb[:, r : r + 1]
                            )

            # V transposes: VhT [e(pair), s] -> Vh [j, e] into vhe (ones col at 64)
            with tc.tile_pool(name="ps_t", bufs=2, space="PSUM") as ps_t:
                for r in range(NPAIR):
                    for jc in range(JCH):
                        tp = ps_t.tile([128, 128], F32, tag="tp")
                        nc.tensor.transpose(
                            tp[:], vhT_tmp[r][:, 128 * jc : 128 * (jc + 1)], ident[:]
                        )
                        for q in range(2):
                            h = 2 * r + q
                            nc.vector.tensor_copy(
                                vhe[h][:, 65 * jc : 65 * jc + 64],
                                tp[:, 64 * q : 64 * (q + 1)],
                            )

        # ---- phase 2: attention ----
        with (
            tc.tile_pool(name="pexp", bufs=3) as pexp_pool,
            tc.tile_pool(name="bc", bufs=2) as bc_pool,
            tc.tile_pool(name="rc", bufs=2) as rc_pool,
            tc.tile_pool(name="ps_sc", bufs=2, space="PSUM") as ps_sc,
            tc.tile_pool(name="ps_acc", bufs=2, space="PSUM") as ps_acc,
        ):
            for h in range(HPG):
                r, q = h // 2, h % 2
                qs = slice(64 * q, 64 * (q + 1))
                acc = [ps_acc.tile([65, IB], F32, tag="acc", name="acc_t") for _ in range(NIB)]
                for jc in range(JCH):
                    k_st = khT[r][qs, 128 * jc : 128 * (jc + 1)]
                    v_st = vhe[h][:, 65 * jc : 65 * (jc + 1)]
                    for ib in range(NIB):
                        s_ps = ps_sc.tile([128, IB], F32, tag="sc")
                        for k in range(IB // 512):
                            i0 = IB * ib + 512 * k
                            nc.tensor.matmul(
                                s_ps[:, 512 * k : 512 * (k + 1)],
                                k_st,
                                qhT[r][qs, i0 : i0 + 512],
                                start=True,
                                stop=True,
                            )
                        pexp = pexp_pool.tile([128, IB], F32R, tag="pexp")
                        nc.scalar.activation(pexp[:], s_ps[:], AF.Exp, scale=INV_SQRT_DK)
                        for k in range(IB // 512):
                            nc.tensor.matmul(
                                acc[ib][:, 512 * k : 512 * (k + 1)],
                                v_st,
                                pexp[:, 512 * k : 512 * (k + 1)],
                                start=(jc == 0),
                                stop=(jc == JCH - 1),
                            )
                for ib in range(NIB):
                    rc = rc_pool.tile([1, IB], F32R, tag="rc")
                    nc.vector.reciprocal(rc[:], acc[ib][64:65, :])
                    bc_ps = ps_sc.tile([128, IB], F32, tag="sc")
                    for k in range(IB // 512):
                        nc.tensor.matmul(
                            bc_ps[0:64, 512 * k : 512 * (k + 1)],
                            ones64[:],
                            rc[:, 512 * k : 512 * (k + 1)],
                            start=True,
                            stop=True,
                        )
                    bc_sb = bc_pool.tile([64, IB], F32, tag="bc")
                    nc.vector.tensor_copy(bc_sb[:], bc_ps[0:64, :])
                    nc.vector.tensor_tensor(
                        outT[r][qs, IB * ib : IB * (ib + 1)],
                        acc[ib][0:64, :],
                        bc_sb[:],
                        mybir.AluOpType.mult,
                    )

        # ---- phase 3: final projection ----
        with (
            tc.tile_pool(name="fo", bufs=4) as fo_pool,
            tc.tile_pool(name="ps_f", bufs=4, space="PSUM") as ps_f,
        ):
            for f in range(D // 128):
                pf = [ps_f.tile([128, 512], F32, tag="pf", name="pf_t") for _ in range(S // 512)]
                for cc in range(2):
                    w_st = wf_sb[:, D * cc + 128 * f : D * cc + 128 * (f + 1)]
                    for i4 in range(S // 512):
                        nc.tensor.matmul(
                            pf[i4][:],
                            w_st,
                            outT[cc][:, 512 * i4 : 512 * (i4 + 1)],
                            start=(cc == 0),
                            stop=(cc == 1),
                        )
                for i4 in range(S // 512):
                    fo = fo_pool.tile([128, 512], F32, tag="fo")
                    nc.vector.tensor_copy(fo[:], pf[i4][:])
                    nc.sync.dma_start(
                        out=pt_d[128 * f : 128 * (f + 1), 512 * i4 : 512 * (i4 + 1)],
                        in_=fo[:],
                    )

    nc.compile()
    return nc


def _get_nc():
    if "nc" not in _CACHE:
        _CACHE["nc"] = _build()
    return _CACHE["nc"]


def kernel(Q, K, V, Wq, bq, Wk, bk, Wv, bv, Wf, bf):
    Q, K, V = np.asarray(Q), np.asarray(K), np.asarray(V)
    Wq, Wk, Wv, Wf = (np.asarray(a) for a in (Wq, Wk, Wv, Wf))
    bq, bk, bv, bf = (np.asarray(a) for a in (bq, bk, bv, bf))

    nc = _get_nc()

    # per-batch transposed inputs, f32r-chopped
    qt = [_chop(Q[b].T) for b in range(B)]
    kt = [_chop(K[b].T) for b in range(B)]
    vt = [_chop(V[b].T) for b in range(B)]
    # per-group weights: [D, CW] with heads side by side, f32r-chopped
    wq_g = [_chop(Wq[HPG * g : HPG * (g + 1)].transpose(1, 0, 2).reshape(D, CW)) for g in range(GPB)]
    wk_g = [_chop(Wk[HPG * g : HPG * (g + 1)].transpose(1, 0, 2).reshape(D, CW)) for g in range(GPB)]
    wv_g = [_chop(Wv[HPG * g : HPG * (g + 1)].transpose(1, 0, 2).reshape(D, CW)) for g in range(GPB)]
    wf_g = [_chop(Wf[CW * g : CW * (g + 1), :]) for g in range(GPB)]
    bq_g = [np.ascontiguousarray(bq[HPG * g : HPG * (g + 1)].reshape(CW), np.float32) for g in range(GPB)]
    bk_g = [np.ascontiguousarray(bk[HPG * g : HPG * (g + 1)].reshape(CW), np.float32) for g in range(GPB)]
    bv_g = [np.ascontiguousarray(bv[HPG * g : HPG * (g + 1)].reshape(CW), np.float32) for g in range(GPB)]

    ones_col = np.ones((128, JCH, 1), np.float32)
    ones_row = np.ones((1, 64), np.float32)
    in_maps = []
    for c in range(NCORES):
        b, g = c // GPB, c % GPB
        in_maps.append(
            {
                "qt": qt[b], "kt": kt[b], "vt": vt[b],
                "wq": wq_g[g], "wk": wk_g[g], "wv": wv_g[g], "wf": wf_g[g],
                "bq": bq_g[g], "bk": bk_g[g], "bv": bv_g[g],
                "ones_col": ones_col, "ones_row": ones_row,
            }
        )

    res = run_bass_kernel_spmd(nc, in_maps, list(range(NCORES)))

    out = np.empty((B, S, D), np.float32)
    bf32 = bf.astype(np.float32)
    for b in range(B):
        acc = res.results[GPB * b]["pt"].astype(np.float32)
        for g in range(1, GPB):
            acc = acc + res.results[GPB * b + g]["pt"]
        out[b] = acc.T + bf32
    return out


# revision 6
# speedup vs baseline: 1.0046x; 1.0046x over previous
"""Multi-head attention (B=2, S=2048, D=1024, H=16, DH=64) on 8 TRN2 cores.

Sharding: core c handles batch b = c//4 and head group g = c%4 (4 heads).
Per core, for its (b, g):
    VhT/KhT/QhT = per-head projections in transposed layout [e, s],
    Vh = PE-transposed back to [j, e] with a ones column appended (vhe),
    S^T = Kh @ Qh^T per head (scores transposed, keys j on partitions),
    P^T = exp(S^T / sqrt(dk))  (no max subtraction; fp32 range is ample),
    acc = Vh_ext^T @ P^T  (row 64 = softmax denominators via the ones col),
    outT = acc[0:64] * (1/l) broadcast  (PE ones-outer-product broadcast),
    PT_partial = sum_c Wf[c,:] outT[c,:]  -> partial final projection [D, S].
Host: out[b] = (sum_g PT_partial).T + bf.

Schedule: V and K stream first (full S), then Q streams in two i-halves;
attention + the final projection for each i-half overlap the later streams.

All matmuls run in float32r (fp32 with low 12 mantissa bits truncated) which
streams at full PE rate (1 cycle/row) vs fp32's 4 cycles/row. Matmul inputs
are pre-truncated on the host and DMA'd into float32r tiles; on-device f32r
producers are DVE/ACT ops with f32r outputs.
"""

import sys

sys.path.insert(0, "/opt/trn_rl_repo")

from contextlib import ExitStack

import numpy as np

import concourse.mybir as mybir
import concourse.tile as tile
from concourse import bacc
from concourse.bass_utils import run_bass_kernel_spmd
from concourse.masks import make_identity

B, S, D, H, DH = 2, 2048, 1024, 16, 64
NCORES = 8
GPB = 4  # head-group cores per batch
HPG = H // GPB  # heads per group (4)
CW = HPG * DH  # concat width per core (256)
NPAIR = HPG // 2  # head pairs per group (2)
DCH = D // 128  # d chunks (8)
JCH = S // 128  # key chunks (16)
IB = 1024  # i-block width for attention
NIB = S // IB  # 2
F32 = mybir.dt.float32
F32R = mybir.dt.float32r
AF = mybir.ActivationFunctionType
INV_SQRT_DK = 1.0 / np.sqrt(DH)

_CACHE = {}


def _chop(x):
    """Truncate fp32 to float32r bit format (low 12 mantissa bits zeroed)."""
    a = np.ascontiguousarray(x, dtype=np.float32)
    return (a.view(np.uint32) & ~np.uint32(0xFFF)).view(np.float32)


def _build():
    nc = bacc.Bacc("TRN2", target_bir_lowering=False, debug=False, num_devices=NCORES)

    qt_d = nc.dram_tensor("qt", [D, S], F32R, kind="ExternalInput").ap()
    kt_d = nc.dram_tensor("kt", [D, S], F32R, kind="ExternalInput").ap()
    vt_d = nc.dram_tensor("vt", [D, S], F32R, kind="ExternalInput").ap()
    wq_d = nc.dram_tensor("wq", [D, CW], F32R, kind="ExternalInput").ap()
    wk_d = nc.dram_tensor("wk", [D, CW], F32R, kind="ExternalInput").ap()
    wv_d = nc.dram_tensor("wv", [D, CW], F32R, kind="ExternalInput").ap()
    wf_d = nc.dram_tensor("wf", [CW, D], F32R, kind="ExternalInput").ap()
    bq_d = nc.dram_tensor("bq", [CW], F32, kind="ExternalInput").ap()
    bk_d = nc.dram_tensor("bk", [CW], F32, kind="ExternalInput").ap()
    bv_d = nc.dram_tensor("bv", [CW], F32, kind="ExternalInput").ap()
    ones_col_d = nc.dram_tensor("ones_col", [128, JCH, 1], F32R, kind="ExternalInput").ap()
    ones_row_d = nc.dram_tensor("ones_row", [1, 64], F32R, kind="ExternalInput").ap()
    pt_d = nc.dram_tensor("pt", [D, S], F32, kind="ExternalOutput").ap()

    with (
        tile.TileContext(nc) as tc,
        nc.allow_low_precision(reason="float32r matmul inputs are intentional"),
        ExitStack() as ctx,
    ):
        const = ctx.enter_context(tc.tile_pool(name="const", bufs=1))
        persist = ctx.enter_context(tc.tile_pool(name="persist", bufs=1))

        # ---- constants (DMA order matters: V-phase needs come first) ----
        wq_sb = const.tile([128, DCH * CW], F32R, tag="wq")
        wk_sb = const.tile([128, DCH * CW], F32R, tag="wk")
        wv_sb = const.tile([128, DCH * CW], F32R, tag="wv")
        wf_sb = const.tile([128, 2 * D], F32R, tag="wf")
        bq_sb = const.tile([128, NPAIR], F32, tag="bq")
        bk_sb = const.tile([128, NPAIR], F32, tag="bk")
        bv_sb = const.tile([128, NPAIR], F32, tag="bv")
        ident = const.tile([128, 128], F32, tag="ident")
        ones64 = const.tile([1, 64], F32R, tag="ones")

        def load_w(w_sb, w_dram):
            nc.sync.dma_start(
                out=w_sb[:].rearrange("p (c e) -> p c e", c=DCH),
                in_=w_dram.rearrange("(c p) e -> p c e", p=128),
            )

        def load_b(b_sb, b_dram):
            nc.sync.dma_start(out=b_sb[:], in_=b_dram.rearrange("(r p) -> p r", p=128))

        # ---- persistent activations ----
        qhT = [persist.tile([128, S], F32R, tag=f"qhT{r}", name=f"qhT{r}") for r in range(NPAIR)]
        khT = [persist.tile([128, S], F32R, tag=f"khT{r}", name=f"khT{r}") for r in range(NPAIR)]
        vhe = [persist.tile([128, JCH * 65], F32R, tag=f"vhe{h}", name=f"vhe{h}") for h in range(HPG)]
        outT = [persist.tile([128, S], F32R, tag=f"outT{r}", name=f"outT{r}") for r in range(NPAIR)]

        make_identity(nc, ident[:])

        # ============ phase 1: V then K projections (full S) ============
        with (
            tc.tile_pool(name="xt", bufs=3) as xt_pool,
            tc.tile_pool(name="vtmp", bufs=1) as vtmp_pool,
        ):
            vhT_tmp = [None, None]
            with tc.tile_pool(name="ps_proj", bufs=2, space="PSUM") as ps_proj:
                # --- V ---
                load_w(wv_sb, wv_d)
                load_b(bv_sb, bv_d)
                for h in range(HPG):
                    nc.sync.dma_start(
                        out=vhe[h][:].rearrange("p (c w) -> p c w", w=65)[:, :, 64:65],
                        in_=ones_col_d,
                    )
                ps = [ps_proj.tile([128, S], F32, tag="proj", name="ps_v") for _ in range(NPAIR)]
                for d in range(DCH):
                    xt = xt_pool.tile([128, S], F32R, tag="xt", name="xt_v")
                    nc.sync.dma_start(out=xt[:], in_=vt_d[128 * d : 128 * (d + 1), :])
                    for r in range(NPAIR):
                        w_st = wv_sb[:, CW * d + 128 * r : CW * d + 128 * (r + 1)]
                        for s4 in range(S // 512):
                            nc.tensor.matmul(
                                ps[r][:, 512 * s4 : 512 * (s4 + 1)],
                                w_st,
                                xt[:, 512 * s4 : 512 * (s4 + 1)],
                                start=(d == 0),
                                stop=(d == DCH - 1),
                            )
                for r in range(NPAIR):
                    vhT_tmp[r] = vtmp_pool.tile([128, S], F32, tag=f"vtmp{r}", name=f"vtmp{r}")
                    nc.vector.tensor_scalar_add(vhT_tmp[r][:], ps[r][:], bv_sb[:, r : r + 1])

                # --- K ---
                load_w(wk_sb, wk_d)
                load_b(bk_sb, bk_d)
                ps = [ps_proj.tile([128, S], F32, tag="proj", name="ps_k") for _ in range(NPAIR)]
                for d in range(DCH):
                    xt = xt_pool.tile([128, S], F32R, tag="xt", name="xt_k")
                    nc.sync.dma_start(out=xt[:], in_=kt_d[128 * d : 128 * (d + 1), :])
                    for r in range(NPAIR):
                        w_st = wk_sb[:, CW * d + 128 * r : CW * d + 128 * (r + 1)]
                        for s4 in range(S // 512):
                            nc.tensor.matmul(
                                ps[r][:, 512 * s4 : 512 * (s4 + 1)],
                                w_st,
                                xt[:, 512 * s4 : 512 * (s4 + 1)],
                                start=(d == 0),
                                stop=(d == DCH - 1),
                            )
                for r in range(NPAIR):
                    nc.vector.tensor_scalar_add(khT[r][:], ps[r][:], bk_sb[:, r : r + 1])

            # V transposes: VhT [e(pair), s] -> Vh [j, e] into vhe (ones col at 64)
            with tc.tile_pool(name="ps_t", bufs=2, space="PSUM") as ps_t:
                for r in range(NPAIR):
                    for jc in range(JCH):
                        tp = ps_t.tile([128, 128], F32, tag="tp", name="tp")
                        nc.tensor.transpose(
                            tp[:], vhT_tmp[r][:, 128 * jc : 128 * (jc + 1)], ident[:]
                        )
                        for q in range(2):
                            h = 2 * r + q
                            nc.vector.tensor_copy(
                                vhe[h][:, 65 * jc : 65 * jc + 64],
                                tp[:, 64 * q : 64 * (q + 1)],
                            )

        # ============ phase 2: Q by i-halves + attention + final ============
        load_w(wq_sb, wq_d)
        load_b(bq_sb, bq_d)
        nc.sync.dma_start(out=ones64[:], in_=ones_row_d)
        nc.sync.dma_start(
            out=wf_sb[:].rearrange("p (c f) -> p c f", c=2),
            in_=wf_d.rearrange("(c p) f -> p c f", p=128),
        )

        with (
            tc.tile_pool(name="qx", bufs=8) as qx_pool,
            tc.tile_pool(name="pexp", bufs=3) as pexp_pool,
            tc.tile_pool(name="bc", bufs=2) as bc_pool,
            tc.tile_pool(name="rc", bufs=2) as rc_pool,
            tc.tile_pool(name="fo", bufs=4) as fo_pool,
            tc.tile_pool(name="ps_sc", bufs=2, space="PSUM") as ps_sc,
            tc.tile_pool(name="ps_acc", bufs=2, space="PSUM") as ps_acc,
        ):
            for ib in range(NIB):
                isl = slice(IB * ib, IB * (ib + 1))
                # --- Q projection for this i-half ---
                ps_q = [
                    ps_sc.tile([128, IB], F32, tag="sc", name="ps_q") for _ in range(NPAIR)
                ]
                for d in range(DCH):
                    qx = qx_pool.tile([128, IB], F32R, tag="qx", name="qx")
                    nc.sync.dma_start(out=qx[:], in_=qt_d[128 * d : 128 * (d + 1), isl])
                    for r in range(NPAIR):
                        w_st = wq_sb[:, CW * d + 128 * r : CW * d + 128 * (r + 1)]
                        for k in range(IB // 512):
                            nc.tensor.matmul(
                                ps_q[r][:, 512 * k : 512 * (k + 1)],
                                w_st,
                                qx[:, 512 * k : 512 * (k + 1)],
                                start=(d == 0),
                                stop=(d == DCH - 1),
                            )
                for r in range(NPAIR):
                    nc.vector.tensor_scalar_add(
                        qhT[r][:, isl], ps_q[r][:], bq_sb[:, r : r + 1]
                    )

                # --- attention for all heads on this i-half ---
                for h in range(HPG):
                    r, q = h // 2, h % 2
                    qs = slice(64 * q, 64 * (q + 1))
                    acc = ps_acc.tile([65, IB], F32, tag="acc", name="acc")
                    for jc in range(JCH):
                        k_st = khT[r][qs, 128 * jc : 128 * (jc + 1)]
                        v_st = vhe[h][:, 65 * jc : 65 * (jc + 1)]
                        s_ps = ps_sc.tile([128, IB], F32, tag="sc", name="s_ps")
                        for k in range(IB // 512):
                            nc.tensor.matmul(
                                s_ps[:, 512 * k : 512 * (k + 1)],
                                k_st,
                                qhT[r][qs, IB * ib + 512 * k : IB * ib + 512 * (k + 1)],
                                start=True,
                                stop=True,
                            )
                        pexp = pexp_pool.tile([128, IB], F32R, tag="pexp", name="pexp")
                        nc.scalar.activation(pexp[:], s_ps[:], AF.Exp, scale=INV_SQRT_DK)
                        for k in range(IB // 512):
                            nc.tensor.matmul(
                                acc[:, 512 * k : 512 * (k + 1)],
                                v_st,
                                pexp[:, 512 * k : 512 * (k + 1)],
                                start=(jc == 0),
                                stop=(jc == JCH - 1),
                            )
                    # normalization
                    rc = rc_pool.tile([1, IB], F32R, tag="rc", name="rc")
                    nc.vector.reciprocal(rc[:], acc[64:65, :])
                    bc_ps = ps_sc.tile([128, IB], F32, tag="sc", name="bc_ps")
                    for k in range(IB // 512):
                        nc.tensor.matmul(
                            bc_ps[0:64, 512 * k : 512 * (k + 1)],
                            ones64[:],
                            rc[:, 512 * k : 512 * (k + 1)],
                            start=True,
                            stop=True,
                        )
                    bc_sb = bc_pool.tile([64, IB], F32, tag="bc", name="bc_sb")
                    nc.vector.tensor_copy(bc_sb[:], bc_ps[0:64, :])
                    nc.vector.tensor_tensor(
                        outT[r][qs, isl],
                        acc[0:64, :],
                        bc_sb[:],
                        mybir.AluOpType.mult,
                    )

                # --- final projection for this i-half ---
                for f in range(D // 128):
                    for i4 in range(IB // 512):
                        i0 = IB * ib + 512 * i4
                        pf = ps_acc.tile([128, 512], F32, tag="acc", name="pf")
                        for cc in range(2):
                            nc.tensor.matmul(
                                pf[:],
                                wf_sb[:, D * cc + 128 * f : D * cc + 128 * (f + 1)],
                                outT[cc][:, i0 : i0 + 512],
                                start=(cc == 0),
                                stop=(cc == 1),
                            )
                        fo = fo_pool.tile([128, 512], F32, tag="fo", name="fo")
                        nc.vector.tensor_copy(fo[:], pf[:])
                        nc.sync.dma_start(
                            out=pt_d[128 * f : 128 * (f + 1), i0 : i0 + 512],
                            in_=fo[:],
                        )

    nc.compile()
    return nc


def _get_nc():
    if "nc" not in _CACHE:
        _CACHE["nc"] = _build()
    return _CACHE["nc"]


def kernel(Q, K, V, Wq, bq, Wk, bk, Wv, bv, Wf, bf):
    Q, K, V = np.asarray(Q), np.asarray(K), np.asarray(V)
    Wq, Wk, Wv, Wf = (np.asarray(a) for a in (Wq, Wk, Wv, Wf))
    bq, bk, bv, bf = (np.asarray(a) for a in (bq, bk, bv, bf))

    nc = _get_nc()

    qt = [_chop(Q[b].T) for b in range(B)]
    kt = [_chop(K[b].T) for b in range(B)]
    vt = [_chop(V[b].T) for b in range(B)]
    wq_g = [_chop(Wq[HPG * g : HPG * (g + 1)].transpose(1, 0, 2).reshape(D, CW)) for g in range(GPB)]
    wk_g = [_chop(Wk[HPG * g : HPG * (g + 1)].transpose(1, 0, 2).reshape(D, CW)) for g in range(GPB)]
    wv_g = [_chop(Wv[HPG * g : HPG * (g + 1)].transpose(1, 0, 2).reshape(D, CW)) for g in range(GPB)]
    wf_g = [_chop(Wf[CW * g : CW * (g + 1), :]) for g in range(GPB)]
    bq_g = [np.ascontiguousarray(bq[HPG * g : HPG * (g + 1)].reshape(CW), np.float32) for g in range(GPB)]
    bk_g = [np.ascontiguousarray(bk[HPG * g : HPG * (g + 1)].reshape(CW), np.float32) for g in range(GPB)]
    bv_g = [np.ascontiguousarray(bv[HPG * g : HPG * (g + 1)].reshape(CW), np.float32) for g in range(GPB)]

    ones_col = np.ones((128, JCH, 1), np.float32)
    ones_row = np.ones((1, 64), np.float32)
    in_maps = []
    for c in range(NCORES):
        b, g = c // GPB, c % GPB
        in_maps.append(
            {
                "qt": qt[b], "kt": kt[b], "vt": vt[b],
                "wq": wq_g[g], "wk": wk_g[g], "wv": wv_g[g], "wf": wf_g[g],
                "bq": bq_g[g], "bk": bk_g[g], "bv": bv_g[g],
                "ones_col": ones_col, "ones_row": ones_row,
            }
        )

    res = run_bass_kernel_spmd(nc, in_maps, list(range(NCORES)))

    out = np.empty((B, S, D), np.float32)
    bf32 = bf.astype(np.float32)
    for b in range(B):
        acc = res.results[GPB * b]["pt"].astype(np.float32)
        for g in range(1, GPB):
            acc = acc + res.results[GPB * b + g]["pt"]
        out[b] = acc.T + bf32
    return out
